# revision 4
# baseline (speedup 1.0000x reference)
"""Causal attention (B=2, T=2048, E=1024, H=16, D=64) on 8 TRN2 NeuronCores.

Sharding: core c handles batch b = c//4 and local head group hg = c%4
(4 heads, 256 head-dims).  Data parallel over batch, tensor parallel over
heads; the output projection is row-parallel, so each core returns a
partial [T, E] output and the host sums the 4 partials per batch.  The
projection bias AND the v-bias contribution (Wp @ bv) are added on the
host: at = PV(v_raw)/denom + bv exactly, so out = out_dev + (Wp@bv + bp).

Device plan (per core, all-bf16 matmuls with fp32 PSUM accumulation):
  phase A (DMA chase): q_t/k_t [hd, t] for head-pair 0 computed directly
    transposed (lhsT=W.T, rhs=xt, contraction over e), ec-outer with 8
    open PSUM groups so matmuls chase the xt DMA arrivals.  DMA order is
    arrival-ordered: scalar ring [wq0, wk0, wq1, wk1, wv], sync ring
    [xt halves, mask, wp], gpsimd ring [bq, bk] + v-ones memsets.
  phase B: v tiles t0..3 (enough for attention ib0), then block-causal
    attention with v4..15 / head-pair-1 q/k / projection interleaved as
    PE fill between attention tiles.
  attention: per-head score tiles st[j, i] (1 PSUM bank each, 3-deep),
    exp on ScalarE (scale=1/8, no max subtraction), causal mask multiply
    on the block diagonal only, PV accumulation over j with 64
    ones-rows giving the softmax denominator replicated on partitions
    0:63; acc pool is 2-deep per head so the next block's PV never
    waits on the reciprocal+normalize drain.
  tail: the last i-block of head-pair 1 is processed in 384+128 column
    chunks; final projections interleave with the 128-chunk and the
    last copies run on ScalarE and DVE concurrently.
"""

import ml_dtypes
import numpy as np

import concourse.bass as bass
import concourse.tile as tile
from concourse import bacc, mybir
from concourse.bass_utils import run_bass_kernel_spmd

B, T, E = 2, 2048, 1024
H, D = 16, 64
NCORES = 8
GROUPS = 4              # cores per batch (tensor parallel over heads)
HL = H // GROUPS        # 4 local heads per core
HDL = HL * D            # 256 local head dims
P = 128
TQ = 512                # i-block (free dim of score tiles)
JB = 128                # j-block (partition dim of score tiles)
N_TB = T // TQ          # 4
N_EC = E // P           # 8
N_TC = T // P           # 16
XH = 1024               # xt SBUF half-tile columns

F32 = mybir.dt.float32
BF16 = mybir.dt.bfloat16
AF = mybir.ActivationFunctionType


def _build_nc():
    nc = bacc.Bacc("TRN2", target_bir_lowering=False, debug=False)
    xt = nc.dram_tensor("xt", [E, T], BF16, kind="ExternalInput").ap()
    wq0 = nc.dram_tensor("wq0", [P, N_EC, P], BF16, kind="ExternalInput").ap()
    wq1 = nc.dram_tensor("wq1", [P, N_EC, P], BF16, kind="ExternalInput").ap()
    wk0 = nc.dram_tensor("wk0", [P, N_EC, P], BF16, kind="ExternalInput").ap()
    wk1 = nc.dram_tensor("wk1", [P, N_EC, P], BF16, kind="ExternalInput").ap()
    wvt = nc.dram_tensor("wvt", [P, N_EC, HDL], BF16, kind="ExternalInput").ap()
    wpt = nc.dram_tensor("wpt", [P, 2, E], BF16, kind="ExternalInput").ap()
    bqv = nc.dram_tensor("bqv", [HDL], F32, kind="ExternalInput").ap()
    bkv = nc.dram_tensor("bkv", [HDL], F32, kind="ExternalInput").ap()
    maskd = nc.dram_tensor("mask", [GROUPS, JB, TQ], BF16,
                           kind="ExternalInput").ap()
    out = nc.dram_tensor("out", [T, E], BF16, kind="ExternalOutput").ap()

    with tile.TileContext(nc) as tc:
        with (
            tc.tile_pool(name="big", bufs=1) as big,
            tc.tile_pool(name="work", bufs=6) as work,
            tc.tile_pool(name="outp", bufs=3) as outp,
        ):
            # ------- input loads, arrival-ordered across 3 rings ----------
            # scalar ring: first-needed weights in consumption order
            wq_h = []
            wk_h = []
            for hc, (wqd, wkd) in enumerate(((wq0, wk0), (wq1, wk1))):
                wq_t = big.tile([P, N_EC, P], BF16, tag=f"wq{hc}",
                                name=f"wq{hc}")
                wk_t = big.tile([P, N_EC, P], BF16, tag=f"wk{hc}",
                                name=f"wk{hc}")
                wq_h.append(wq_t)
                wk_h.append(wk_t)
            nc.scalar.dma_start(wq_h[0], wq0)
            nc.scalar.dma_start(wk_h[0], wk0)
            nc.scalar.dma_start(wq_h[1], wq1)
            nc.scalar.dma_start(wk_h[1], wk1)
            wv_all = big.tile([P, N_EC, HDL], BF16, tag="wv", name="wv")
            nc.scalar.dma_start(wv_all, wvt)
            # sync ring: xt halves (16 x 256KB), then mask + wp (needed late)
            xt_sb = [big.tile([P, XH], BF16, tag=f"xt{i}", name=f"xt{i}")
                     for i in range(2 * N_EC)]
            for ec in range(N_EC):
                for hf in range(2):
                    nc.sync.dma_start(
                        xt_sb[2 * ec + hf],
                        xt[ec * P:(ec + 1) * P, hf * XH:(hf + 1) * XH])
            mask_sb = big.tile([P, GROUPS, TQ], BF16, tag="mask", name="mask")
            nc.sync.dma_start(mask_sb, maskd.rearrange("d p f -> p d f"))
            wp_all = big.tile([P, 2, E], BF16, tag="wp", name="wp")
            nc.sync.dma_start(wp_all, wpt)
            # gpsimd ring: tiny biases, then v ones-region memsets
            bq_sb = big.tile([P, 2], F32, tag="bq", name="bq")
            nc.gpsimd.dma_start(bq_sb, bqv.rearrange("(c p) -> p c", p=P))
            bk_sb = big.tile([P, 2], F32, tag="bk", name="bk")
            nc.gpsimd.dma_start(bk_sb, bkv.rearrange("(c p) -> p c", p=P))

            q_sb = [big.tile([P, T], BF16, tag=f"q{hc}", name=f"q{hc}")
                    for hc in range(2)]
            k_sb = [big.tile([P, T], BF16, tag=f"k{hc}", name=f"k{hc}")
                    for hc in range(2)]
            at_sb = [big.tile([P, T], BF16, tag=f"at{hc}", name=f"at{hc}")
                     for hc in range(2)]
            v_sb = [big.tile([P, HL, 2 * D], BF16, tag=f"v{t}", name=f"v{t}")
                    for t in range(N_TC)]
            for t_ in range(N_TC):
                nc.gpsimd.memset(v_sb[t_][:, :, 0:D], 1.0)

            def xt_cols(ec, c0, c1):
                hf, off = divmod(c0, XH)
                assert c1 - c0 <= XH - off
                return xt_sb[2 * ec + hf][:, off:off + (c1 - c0)]

            # ------- phase A (bf16): q/k for head-pair 0; ec-outer ---------
            def qk_phase(ph2ps, hc):
                pss = [ph2ps.tile([P, TQ], F32, tag="mm", name="mm")
                       for _ in range(8)]
                for ec in range(N_EC):
                    for tb in range(N_TB):
                        for wi, w_all in enumerate((wq_h, wk_h)):
                            nc.tensor.matmul(
                                pss[tb * 2 + wi],
                                lhsT=w_all[hc][:, ec, :],
                                rhs=xt_cols(ec, tb * TQ, (tb + 1) * TQ),
                                start=(ec == 0), stop=(ec == N_EC - 1))
                for tb in range(N_TB):
                    for wi, (bias_t, dst) in enumerate(((bq_sb, q_sb),
                                                        (bk_sb, k_sb))):
                        nc.vector.tensor_scalar_add(
                            dst[hc][:, tb * TQ:(tb + 1) * TQ],
                            pss[tb * 2 + wi], bias_t[:, hc:hc + 1])

            def v_wave(ph2ps, ts):
                pss = [ph2ps.tile([P, HDL], F32, tag="mm", name="mm")
                       for _ in ts]
                for ec in range(N_EC):
                    for i, t_ in enumerate(ts):
                        nc.tensor.matmul(
                            pss[i],
                            lhsT=xt_cols(ec, t_ * P, (t_ + 1) * P),
                            rhs=wv_all[:, ec, :],
                            start=(ec == 0), stop=(ec == N_EC - 1))
                for i, t_ in enumerate(ts):
                    nc.vector.tensor_copy(
                        v_sb[t_][:, :, D:2 * D],
                        pss[i].rearrange("p (h d) -> p h d", h=HL))

            # ------- phase 3 (bf16): block-causal attention ----------------
            # processes i-columns [ib*TQ+i0, ib*TQ+i1) for head pair hp
            def attention(stps, accps, hp, ib, i0=0, i1=TQ, filler=None):
                base = ib * TQ + i0
                L = i1 - i0
                njb = (ib * TQ + i1) // JB
                accs = [accps.tile([P, TQ], F32, tag=f"acc{h}",
                                   name=f"acc{h}") for h in range(2)]
                for jb in range(njb):
                    d = (jb * JB - base) // JB      # >= 0 on block diagonal
                    dd = max(0, jb * JB - base)     # local masked-col trim
                    pts = []
                    for h in range(2):
                        pr = slice(h * D, (h + 1) * D)
                        st = stps.tile([P, TQ], F32, tag="st", name="st")
                        pt = work.tile([P, TQ], BF16, tag="pt", name="pt")
                        pts.append(pt)
                        nc.tensor.matmul(
                            st[:, dd:L],
                            lhsT=q_sb[hp][pr, jb * JB:(jb + 1) * JB],
                            rhs=k_sb[hp][pr, base + dd:base + L],
                            start=True, stop=True)
                        nc.scalar.activation(pt[:, dd:L], st[:, dd:L],
                                             AF.Exp, scale=0.125)
                        if d >= 0:
                            nc.vector.tensor_mul(
                                pt[:, dd:L], pt[:, dd:L],
                                mask_sb[:, d, dd:L])
                    for h in range(2):
                        nc.tensor.matmul(
                            accs[h][:, dd:L],
                            lhsT=v_sb[jb][:, 2 * hp + h, :],
                            rhs=pts[h][:, dd:L],
                            start=(jb == 0), stop=(jb == njb - 1))
                    if filler and jb % 2 == 1:
                        filler.pop(0)()
                # normalize in <=256-col chunks (faster acc drain)
                for h in range(2):
                    for c0 in range(0, L, 256):
                        c1 = min(L, c0 + 256)
                        rec = work.tile([D, 256], F32, tag="rec", name="rec")
                        nc.vector.reciprocal_approx_fast(
                            rec[:, :c1 - c0], accs[h][0:D, c0:c1])
                        nc.vector.tensor_mul(
                            at_sb[hp][h * D:(h + 1) * D,
                                      base + c0:base + c1],
                            accs[h][D:2 * D, c0:c1], rec[:, :c1 - c0])

            # ------- fillers: hc1 q/k, v waves, projection -----------------
            def qk2_group(mmps, tb, wi):
                w_all = (wq_h, wk_h)[wi]
                bias_t = (bq_sb, bk_sb)[wi]
                dst = (q_sb, k_sb)[wi]

                def go():
                    ps = mmps.tile([P, TQ], F32, tag="mm", name="mm")
                    for ec in range(N_EC):
                        nc.tensor.matmul(
                            ps,
                            lhsT=w_all[1][:, ec, :],
                            rhs=xt_cols(ec, tb * TQ, (tb + 1) * TQ),
                            start=(ec == 0), stop=(ec == N_EC - 1))
                    nc.vector.tensor_scalar_add(
                        dst[1][:, tb * TQ:(tb + 1) * TQ], ps, bias_t[:, 1:2])
                return go

            def v_group(mmps, t_):
                def go():
                    ps = mmps.tile([P, HDL], F32, tag="mm", name="mm")
                    for ec in range(N_EC):
                        nc.tensor.matmul(
                            ps,
                            lhsT=xt_cols(ec, t_ * P, (t_ + 1) * P),
                            rhs=wv_all[:, ec, :],
                            start=(ec == 0), stop=(ec == N_EC - 1))
                    nc.vector.tensor_copy(
                        v_sb[t_][:, :, D:2 * D],
                        ps.rearrange("p (h d) -> p h d", h=HL))
                return go

            def proj_t(mmps, t_, copy_eng=None):
                def go():
                    ot = outp.tile([P, E], BF16, tag="ot", name="ot")
                    for eb in range(2):
                        ps = mmps.tile([P, TQ], F32, tag="mm", name="mm")
                        for hc in range(2):
                            nc.tensor.matmul(
                                ps,
                                lhsT=at_sb[hc][:, t_ * P:(t_ + 1) * P],
                                rhs=wp_all[:, hc, eb * TQ:(eb + 1) * TQ],
                                start=(hc == 0), stop=(hc == 1))
                        if copy_eng is not None and eb == 0:
                            nc.scalar.copy(ot[:, eb * TQ:(eb + 1) * TQ], ps)
                        else:
                            nc.vector.tensor_copy(
                                ot[:, eb * TQ:(eb + 1) * TQ], ps)
                    nc.sync.dma_start(out[t_ * P:(t_ + 1) * P, :], ot)
                return go

            # ------- orchestration -----------------------------------------
            import contextlib
            with tc.tile_pool(name="ph2ps", bufs=8, space="PSUM") as ph2ps:
                qk_phase(ph2ps, 0)
                v_wave(ph2ps, ts=(0, 1, 2, 3))
            _ph34 = contextlib.ExitStack()
            stps = _ph34.enter_context(
                tc.tile_pool(name="stps", bufs=3, space="PSUM"))
            accps = _ph34.enter_context(
                tc.tile_pool(name="accps", bufs=2, space="PSUM"))
            mmps = _ph34.enter_context(
                tc.tile_pool(name="mmps", bufs=1, space="PSUM"))

            def F(*gs):
                return list(gs)

            # interleave the two head pairs at i-block granularity; fillers
            # are emitted between attention j-tiles (1 per 2 tiles).
            # Emission order is program order on each in-order engine queue,
            # so every filler must be emitted after its producers.
            attention(stps, accps, 0, 0,
                      filler=F(v_group(mmps, 4), v_group(mmps, 5)))
            qk2_group(mmps, 0, 0)()
            qk2_group(mmps, 0, 1)()
            attention(stps, accps, 1, 0,
                      filler=F(v_group(mmps, 6), v_group(mmps, 7)))
            attention(stps, accps, 0, 1,
                      filler=F(qk2_group(mmps, 1, 0), qk2_group(mmps, 1, 1),
                               v_group(mmps, 8), v_group(mmps, 9)))
            attention(stps, accps, 1, 1,
                      filler=F(proj_t(mmps, 0), proj_t(mmps, 1),
                               proj_t(mmps, 2), proj_t(mmps, 3)))
            attention(stps, accps, 0, 2,
                      filler=F(qk2_group(mmps, 2, 0), qk2_group(mmps, 2, 1),
                               v_group(mmps, 10), v_group(mmps, 11),
                               v_group(mmps, 12), v_group(mmps, 13)))
            attention(stps, accps, 1, 2,
                      filler=F(v_group(mmps, 14), v_group(mmps, 15),
                               proj_t(mmps, 4), proj_t(mmps, 5),
                               proj_t(mmps, 6), proj_t(mmps, 7)))
            attention(stps, accps, 0, 3,
                      filler=F(qk2_group(mmps, 3, 0), qk2_group(mmps, 3, 1),
                               proj_t(mmps, 8), proj_t(mmps, 9),
                               proj_t(mmps, 10), proj_t(mmps, 11)))
            # last block of hp1 in 384+128 column chunks so the tail chain
            # (norm -> proj -> copy -> DMA) only covers 128 columns.
            attention(stps, accps, 1, 3, i0=0, i1=384)
            proj_t(mmps, 12)()
            proj_t(mmps, 13)()
            attention(stps, accps, 1, 3, i0=384, i1=TQ,
                      filler=F(proj_t(mmps, 14, copy_eng="scalar")))
            proj_t(mmps, 15, copy_eng="scalar")()
            _ph34.close()

    nc.compile()
    return nc


def _make_mask():
    jj = np.arange(JB)[:, None]
    ii = np.arange(TQ)[None, :]
    m = np.zeros((GROUPS, JB, TQ), dtype=np.float32)
    for d in range(GROUPS):
        m[d] = (jj + d * JB <= ii).astype(np.float32)
    return m.astype(ml_dtypes.bfloat16)


_NC = None


def _get_nc():
    global _NC
    if _NC is None:
        _NC = _build_nc()
    return _NC


def _warr(w):
    """W slice [HDL, E] -> SBUF layout [P, N_EC, HDL]: element (p, c, f) =
    W.T[c*P + p, f]."""
    return np.ascontiguousarray(
        w.T.reshape(N_EC, P, HDL).transpose(1, 0, 2)).astype(ml_dtypes.bfloat16)


def kernel(x, Wq, bq, Wk, bk, Wv, bv, Wp, bp, **_run_kwargs):
    x = np.asarray(x, dtype=np.float32)
    Wq = np.asarray(Wq, dtype=np.float32)
    Wk = np.asarray(Wk, dtype=np.float32)
    Wv = np.asarray(Wv, dtype=np.float32)
    Wp = np.asarray(Wp, dtype=np.float32)
    bq = np.asarray(bq, dtype=np.float32)
    bk = np.asarray(bk, dtype=np.float32)
    bv = np.asarray(bv, dtype=np.float32)
    bp = np.asarray(bp, dtype=np.float32)

    mask = _make_mask()
    # at = PV(v_raw)/denom + bv exactly, so the v-bias and projection bias
    # fold into one host-side vector: out += Wp @ bv + bp.
    bias_eff = (bp + Wp @ bv).astype(np.float32)

    in_maps = []
    for c in range(NCORES):
        b, hg = divmod(c, GROUPS)
        hsl = slice(HDL * hg, HDL * (hg + 1))
        wq_a = _warr(Wq[hsl])
        wk_a = _warr(Wk[hsl])
        in_maps.append({
            "xt": np.ascontiguousarray(x[b].T).astype(ml_dtypes.bfloat16),
            "wq0": np.ascontiguousarray(wq_a[:, :, 0:P]),
            "wq1": np.ascontiguousarray(wq_a[:, :, P:2 * P]),
            "wk0": np.ascontiguousarray(wk_a[:, :, 0:P]),
            "wk1": np.ascontiguousarray(wk_a[:, :, P:2 * P]),
            "wvt": _warr(Wv[hsl]),
            "wpt": np.ascontiguousarray(
                Wp[:, hsl].T.reshape(2, P, E).transpose(1, 0, 2)
            ).astype(ml_dtypes.bfloat16),
            "bqv": np.ascontiguousarray(bq[hsl]),
            "bkv": np.ascontiguousarray(bk[hsl]),
            "mask": mask,
        })

    nc = _get_nc()
    try:
        res = run_bass_kernel_spmd(nc, in_maps, core_ids=list(range(NCORES)),
                                   **_run_kwargs)
    except Exception:
        # transient device hiccups (e.g. NRT_EXEC_UNIT_UNRECOVERABLE) have
        # been observed to clear on retry
        import time
        time.sleep(2.0)
        res = run_bass_kernel_spmd(nc, in_maps, core_ids=list(range(NCORES)),
                                   **_run_kwargs)
    outs = [r["out"].astype(np.float32) for r in res.results]
    y = np.stack([
        outs[0] + outs[1] + outs[2] + outs[3] + bias_eff,
        outs[4] + outs[5] + outs[6] + outs[7] + bias_eff,
    ]).astype(np.float32)
    if _run_kwargs:
        return y, res
    return y


# revision 9
# speedup vs baseline: 1.1063x; 1.1063x over previous
"""Causal attention (B=2, T=2048, E=1024, H=16, D=64) on 8 TRN2 NeuronCores.

Sharding: core c handles batch b = c//4 and local head group hg = c%4
(4 heads, 256 head-dims).  Data parallel over batch, tensor parallel over
heads; the output projection is row-parallel, so each core returns a
partial [T, E] output and the host sums the 4 partials per batch.  The
projection bias AND the v-bias contribution (Wp @ bv) are added on the
host: at = PV(v_raw)/denom + bv exactly, so out = out_dev + (Wp@bv + bp).

Device plan (per core, all-bf16 matmuls with fp32 PSUM accumulation):
  phase A (DMA chase): q_t/k_t [hd, t] for head-pair 0 computed directly
    transposed (lhsT=W.T, rhs=xt, contraction over e), ec-outer with 8
    open PSUM groups so matmuls chase the xt DMA arrivals.  DMA order is
    arrival-ordered: scalar ring [wq0, wk0, wq1, wk1, wv], sync ring
    [xt halves, mask, wp], gpsimd ring [bq, bk] + v-ones memsets.
  phase B: v tiles t0..3 (enough for attention ib0), then block-causal
    attention with v4..15 / head-pair-1 q/k / projection interleaved as
    PE fill between attention tiles.
  attention: per-head score tiles st[j, i] (1 PSUM bank each, 3-deep),
    exp on ScalarE (scale=1/8, no max subtraction), causal mask multiply
    on the block diagonal only, PV accumulation over j with 64
    ones-rows giving the softmax denominator replicated on partitions
    0:63; acc pool is 2-deep per head so the next block's PV never
    waits on the reciprocal+normalize drain.
  tail: the last i-block of head-pair 1 is processed in 384+128 column
    chunks; final projections interleave with the 128-chunk and the
    last copies run on ScalarE and DVE concurrently.
"""

import ml_dtypes
import numpy as np

import concourse.bass as bass
import concourse.tile as tile
from concourse import bacc, mybir
from concourse.bass_utils import run_bass_kernel_spmd

B, T, E = 2, 2048, 1024
H, D = 16, 64
NCORES = 8
GROUPS = 4              # cores per batch (tensor parallel over heads)
HL = H // GROUPS        # 4 local heads per core
HDL = HL * D            # 256 local head dims
P = 128
TQ = 512                # i-block (free dim of score tiles)
JB = 128                # j-block (partition dim of score tiles)
N_TB = T // TQ          # 4
N_EC = E // P           # 8
N_TC = T // P           # 16
XH = 1024               # xt SBUF half-tile columns

F32 = mybir.dt.float32
BF16 = mybir.dt.bfloat16
AF = mybir.ActivationFunctionType


def _build_nc():
    nc = bacc.Bacc("TRN2", target_bir_lowering=False, debug=False)
    xt = nc.dram_tensor("xt", [E, T], BF16, kind="ExternalInput").ap()
    wq0 = nc.dram_tensor("wq0", [P, N_EC, P], BF16, kind="ExternalInput").ap()
    wq1 = nc.dram_tensor("wq1", [P, N_EC, P], BF16, kind="ExternalInput").ap()
    wk0 = nc.dram_tensor("wk0", [P, N_EC, P], BF16, kind="ExternalInput").ap()
    wk1 = nc.dram_tensor("wk1", [P, N_EC, P], BF16, kind="ExternalInput").ap()
    wvt = nc.dram_tensor("wvt", [P, N_EC, HDL], BF16, kind="ExternalInput").ap()
    wpt = nc.dram_tensor("wpt", [P, 2, E], BF16, kind="ExternalInput").ap()
    bqv = nc.dram_tensor("bqv", [HDL], F32, kind="ExternalInput").ap()
    bkv = nc.dram_tensor("bkv", [HDL], F32, kind="ExternalInput").ap()
    maskd = nc.dram_tensor("mask", [GROUPS, JB, TQ], BF16,
                           kind="ExternalInput").ap()
    out = nc.dram_tensor("out", [T, E], BF16, kind="ExternalOutput").ap()

    with tile.TileContext(nc) as tc:
        with (
            tc.tile_pool(name="big", bufs=1) as big,
            tc.tile_pool(name="work", bufs=6) as work,
            tc.tile_pool(name="outp", bufs=3) as outp,
        ):
            # ------- input loads, arrival-ordered across 3 rings ----------
            # scalar ring: first-needed weights in consumption order
            wq_h = []
            wk_h = []
            for hc, (wqd, wkd) in enumerate(((wq0, wk0), (wq1, wk1))):
                wq_t = big.tile([P, N_EC, P], BF16, tag=f"wq{hc}",
                                name=f"wq{hc}")
                wk_t = big.tile([P, N_EC, P], BF16, tag=f"wk{hc}",
                                name=f"wk{hc}")
                wq_h.append(wq_t)
                wk_h.append(wk_t)
            nc.scalar.dma_start(wq_h[0], wq0)
            nc.scalar.dma_start(wk_h[0], wk0)
            nc.scalar.dma_start(wq_h[1], wq1)
            nc.scalar.dma_start(wk_h[1], wk1)
            wv_all = big.tile([P, N_EC, HDL], BF16, tag="wv", name="wv")
            nc.scalar.dma_start(wv_all, wvt)
            # sync ring: xt halves (16 x 256KB), then mask + wp (needed late)
            xt_sb = [big.tile([P, XH], BF16, tag=f"xt{i}", name=f"xt{i}")
                     for i in range(2 * N_EC)]
            for ec in range(N_EC):
                for hf in range(2):
                    nc.sync.dma_start(
                        xt_sb[2 * ec + hf],
                        xt[ec * P:(ec + 1) * P, hf * XH:(hf + 1) * XH])
            mask_sb = big.tile([P, GROUPS, TQ], BF16, tag="mask", name="mask")
            nc.sync.dma_start(mask_sb, maskd.rearrange("d p f -> p d f"))
            wp_all = big.tile([P, 2, E], BF16, tag="wp", name="wp")
            nc.sync.dma_start(wp_all, wpt)
            # gpsimd ring: tiny biases, then v ones-region memsets
            bq_sb = big.tile([P, 2], F32, tag="bq", name="bq")
            nc.gpsimd.dma_start(bq_sb, bqv.rearrange("(c p) -> p c", p=P))
            bk_sb = big.tile([P, 2], F32, tag="bk", name="bk")
            nc.gpsimd.dma_start(bk_sb, bkv.rearrange("(c p) -> p c", p=P))

            q_sb = [big.tile([P, T], BF16, tag=f"q{hc}", name=f"q{hc}")
                    for hc in range(2)]
            k_sb = [big.tile([P, T], BF16, tag=f"k{hc}", name=f"k{hc}")
                    for hc in range(2)]
            at_sb = [big.tile([P, T], BF16, tag=f"at{hc}", name=f"at{hc}")
                     for hc in range(2)]
            v_sb = [big.tile([P, HL, 2 * D], BF16, tag=f"v{t}", name=f"v{t}")
                    for t in range(N_TC)]
            for t_ in range(N_TC):
                nc.gpsimd.memset(v_sb[t_][:, :, 0:D], 1.0)

            def xt_cols(ec, c0, c1):
                hf, off = divmod(c0, XH)
                assert c1 - c0 <= XH - off
                return xt_sb[2 * ec + hf][:, off:off + (c1 - c0)]

            # ------- phase A (bf16): q/k for head-pair 0; ec-outer ---------
            def qk_phase(ph2ps, hc):
                pss = [ph2ps.tile([P, TQ], F32, tag="mm", name="mm")
                       for _ in range(8)]
                for ec in range(N_EC):
                    for tb in range(N_TB):
                        for wi, w_all in enumerate((wq_h, wk_h)):
                            nc.tensor.matmul(
                                pss[tb * 2 + wi],
                                lhsT=w_all[hc][:, ec, :],
                                rhs=xt_cols(ec, tb * TQ, (tb + 1) * TQ),
                                start=(ec == 0), stop=(ec == N_EC - 1))
                for tb in range(N_TB):
                    for wi, (bias_t, dst) in enumerate(((bq_sb, q_sb),
                                                        (bk_sb, k_sb))):
                        nc.vector.tensor_scalar_add(
                            dst[hc][:, tb * TQ:(tb + 1) * TQ],
                            pss[tb * 2 + wi], bias_t[:, hc:hc + 1])

            def v_wave(ph2ps, ts):
                pss = [ph2ps.tile([P, HDL], F32, tag="mm", name="mm")
                       for _ in ts]
                for ec in range(N_EC):
                    for i, t_ in enumerate(ts):
                        nc.tensor.matmul(
                            pss[i],
                            lhsT=xt_cols(ec, t_ * P, (t_ + 1) * P),
                            rhs=wv_all[:, ec, :],
                            start=(ec == 0), stop=(ec == N_EC - 1))
                for i, t_ in enumerate(ts):
                    nc.vector.tensor_copy(
                        v_sb[t_][:, :, D:2 * D],
                        pss[i].rearrange("p (h d) -> p h d", h=HL))

            # ------- phase 3 (bf16): block-causal attention ----------------
            # One [P, 2, TQ] score tile per j-tile covers both heads: the
            # second K=64 score matmul then carries no PSUM-bank wait, so it
            # issues back-to-back with the first and the PE runs the two
            # 64-row quadrants concurrently.
            def norm_chunk(accs, hp, ib, c0, c1):
                base = ib * TQ
                for h in range(2):
                    rec = work.tile([D, 256], F32, tag="rec", name="rec")
                    nc.vector.reciprocal_approx_fast(
                        rec[:, :c1 - c0], accs[h][0:D, c0:c1])
                    nc.vector.tensor_mul(
                        at_sb[hp][h * D:(h + 1) * D, base + c0:base + c1],
                        accs[h][D:2 * D, c0:c1], rec[:, :c1 - c0])

            def attention(stps, accps, hp, ib, filler=None, norm=True):
                base = ib * TQ
                njb = 4 * ib + 4
                accs = [accps.tile([P, TQ], F32, tag=f"acc{h}",
                                   name=f"acc{h}") for h in range(2)]
                for jb in range(njb):
                    d = jb - 4 * ib                 # >= 0 on block diagonal
                    dd = d * JB if d >= 0 else 0    # local masked-col trim
                    st = stps.tile([P, 2, TQ], F32, tag="st", name="st")
                    pt = work.tile([P, 2, TQ], BF16, tag="pt", name="pt")
                    for h in range(2):
                        pr = slice(h * D, (h + 1) * D)
                        nc.tensor.matmul(
                            st[:, h, dd:],
                            lhsT=q_sb[hp][pr, jb * JB:(jb + 1) * JB],
                            rhs=k_sb[hp][pr, base + dd:base + TQ],
                            start=True, stop=True)
                    nc.scalar.activation(pt[:, :, dd:], st[:, :, dd:],
                                         AF.Exp, scale=0.125)
                    if d >= 0:
                        for h in range(2):
                            nc.vector.tensor_mul(
                                pt[:, h, dd:], pt[:, h, dd:],
                                mask_sb[:, d, dd:])
                    for h in range(2):
                        nc.tensor.matmul(
                            accs[h][:, dd:],
                            lhsT=v_sb[jb][:, 2 * hp + h, :],
                            rhs=pt[:, h, dd:],
                            start=(jb == 0), stop=(jb == njb - 1))
                    if filler and jb % 2 == 1:
                        filler.pop(0)()
                if norm:
                    for c0 in range(0, TQ, 256):
                        norm_chunk(accs, hp, ib, c0, c0 + 256)
                return accs

            # ------- fillers: hc1 q/k, v waves, projection -----------------
            def qk2_group(mmps, tb, wi):
                w_all = (wq_h, wk_h)[wi]
                bias_t = (bq_sb, bk_sb)[wi]
                dst = (q_sb, k_sb)[wi]

                def go():
                    ps = mmps.tile([P, TQ], F32, tag="mm", name="mm")
                    for ec in range(N_EC):
                        nc.tensor.matmul(
                            ps,
                            lhsT=w_all[1][:, ec, :],
                            rhs=xt_cols(ec, tb * TQ, (tb + 1) * TQ),
                            start=(ec == 0), stop=(ec == N_EC - 1))
                    nc.vector.tensor_scalar_add(
                        dst[1][:, tb * TQ:(tb + 1) * TQ], ps, bias_t[:, 1:2])
                return go

            def v_group(mmps, t_):
                def go():
                    ps = mmps.tile([P, HDL], F32, tag="mm", name="mm")
                    for ec in range(N_EC):
                        nc.tensor.matmul(
                            ps,
                            lhsT=xt_cols(ec, t_ * P, (t_ + 1) * P),
                            rhs=wv_all[:, ec, :],
                            start=(ec == 0), stop=(ec == N_EC - 1))
                    nc.vector.tensor_copy(
                        v_sb[t_][:, :, D:2 * D],
                        ps.rearrange("p (h d) -> p h d", h=HL))
                return go

            def proj_t(mmps, t_, copy_eng=None):
                # copy_eng: None -> both PSUM drains on DVE; "scalar" -> both
                # on ScalarE; "mix" -> eb0 on ScalarE, eb1 on DVE (parallel).
                def go():
                    ot = outp.tile([P, E], BF16, tag="ot", name="ot")
                    for eb in range(2):
                        ps = mmps.tile([P, TQ], F32, tag="mm", name="mm")
                        for hc in range(2):
                            nc.tensor.matmul(
                                ps,
                                lhsT=at_sb[hc][:, t_ * P:(t_ + 1) * P],
                                rhs=wp_all[:, hc, eb * TQ:(eb + 1) * TQ],
                                start=(hc == 0), stop=(hc == 1))
                        on_scalar = (copy_eng == "scalar"
                                     or (copy_eng == "mix" and eb == 0))
                        if on_scalar:
                            nc.scalar.copy(ot[:, eb * TQ:(eb + 1) * TQ], ps)
                        else:
                            nc.vector.tensor_copy(
                                ot[:, eb * TQ:(eb + 1) * TQ], ps)
                    nc.sync.dma_start(out[t_ * P:(t_ + 1) * P, :], ot)
                return go

            # ------- orchestration -----------------------------------------
            import contextlib
            with tc.tile_pool(name="ph2ps", bufs=8, space="PSUM") as ph2ps:
                qk_phase(ph2ps, 0)
                v_wave(ph2ps, ts=(0, 1, 2, 3))
            _ph34 = contextlib.ExitStack()
            stps = _ph34.enter_context(
                tc.tile_pool(name="stps", bufs=2, space="PSUM"))
            accps = _ph34.enter_context(
                tc.tile_pool(name="accps", bufs=1, space="PSUM"))
            mmps = _ph34.enter_context(
                tc.tile_pool(name="mmps", bufs=2, space="PSUM"))

            def F(*gs):
                return list(gs)

            # interleave the two head pairs at i-block granularity; fillers
            # are emitted between attention j-tiles (1 per 2 tiles) AND at
            # every block boundary, where they hide the acc-drain (norm)
            # latency of the previous block.
            # Emission order is program order on each in-order engine queue,
            # so every filler must be emitted after its producers.
            attention(stps, accps, 0, 0,
                      filler=F(v_group(mmps, 4), v_group(mmps, 5)))
            qk2_group(mmps, 0, 0)()
            qk2_group(mmps, 0, 1)()
            attention(stps, accps, 1, 0,
                      filler=F(v_group(mmps, 6), v_group(mmps, 7)))
            qk2_group(mmps, 1, 0)()
            attention(stps, accps, 0, 1,
                      filler=F(qk2_group(mmps, 1, 1),
                               v_group(mmps, 8), v_group(mmps, 9)))
            v_group(mmps, 10)()
            attention(stps, accps, 1, 1,
                      filler=F(proj_t(mmps, 0), proj_t(mmps, 1),
                               proj_t(mmps, 2), proj_t(mmps, 3)))
            qk2_group(mmps, 2, 0)()
            attention(stps, accps, 0, 2,
                      filler=F(qk2_group(mmps, 2, 1),
                               v_group(mmps, 11), v_group(mmps, 12),
                               v_group(mmps, 13), v_group(mmps, 14)))
            v_group(mmps, 15)()
            attention(stps, accps, 1, 2,
                      filler=F(proj_t(mmps, 4), proj_t(mmps, 5),
                               proj_t(mmps, 6), proj_t(mmps, 7),
                               qk2_group(mmps, 3, 0)))
            proj_t(mmps, 8)()
            attention(stps, accps, 0, 3,
                      filler=F(qk2_group(mmps, 3, 1), proj_t(mmps, 9)))
            proj_t(mmps, 10)()
            accs_last = attention(stps, accps, 1, 3, norm=False,
                                  filler=F(proj_t(mmps, 11)))
            # tail: interleave the last block's norm chunks with the final
            # projections so the serial chain only covers 256 columns.
            norm_chunk(accs_last, 1, 3, 0, 256)
            proj_t(mmps, 12, copy_eng="scalar")()
            norm_chunk(accs_last, 1, 3, 256, TQ)
            proj_t(mmps, 13, copy_eng="scalar")()
            proj_t(mmps, 14, copy_eng="mix")()
            proj_t(mmps, 15, copy_eng="mix")()
            _ph34.close()

    nc.compile()
    return nc


def _make_mask():
    jj = np.arange(JB)[:, None]
    ii = np.arange(TQ)[None, :]
    m = np.zeros((GROUPS, JB, TQ), dtype=np.float32)
    for d in range(GROUPS):
        m[d] = (jj + d * JB <= ii).astype(np.float32)
    return m.astype(ml_dtypes.bfloat16)


_NC = None


def _get_nc():
    global _NC
    if _NC is None:
        _NC = _build_nc()
    return _NC


def _warr(w):
    """W slice [HDL, E] -> SBUF layout [P, N_EC, HDL]: element (p, c, f) =
    W.T[c*P + p, f]."""
    return np.ascontiguousarray(
        w.T.reshape(N_EC, P, HDL).transpose(1, 0, 2)).astype(ml_dtypes.bfloat16)


def kernel(x, Wq, bq, Wk, bk, Wv, bv, Wp, bp, **_run_kwargs):
    x = np.asarray(x, dtype=np.float32)
    Wq = np.asarray(Wq, dtype=np.float32)
    Wk = np.asarray(Wk, dtype=np.float32)
    Wv = np.asarray(Wv, dtype=np.float32)
    Wp = np.asarray(Wp, dtype=np.float32)
    bq = np.asarray(bq, dtype=np.float32)
    bk = np.asarray(bk, dtype=np.float32)
    bv = np.asarray(bv, dtype=np.float32)
    bp = np.asarray(bp, dtype=np.float32)

    mask = _make_mask()
    # at = PV(v_raw)/denom + bv exactly, so the v-bias and projection bias
    # fold into one host-side vector: out += Wp @ bv + bp.
    bias_eff = (bp + Wp @ bv).astype(np.float32)

    in_maps = []
    for c in range(NCORES):
        b, hg = divmod(c, GROUPS)
        hsl = slice(HDL * hg, HDL * (hg + 1))
        wq_a = _warr(Wq[hsl])
        wk_a = _warr(Wk[hsl])
        in_maps.append({
            "xt": np.ascontiguousarray(x[b].T).astype(ml_dtypes.bfloat16),
            "wq0": np.ascontiguousarray(wq_a[:, :, 0:P]),
            "wq1": np.ascontiguousarray(wq_a[:, :, P:2 * P]),
            "wk0": np.ascontiguousarray(wk_a[:, :, 0:P]),
            "wk1": np.ascontiguousarray(wk_a[:, :, P:2 * P]),
            "wvt": _warr(Wv[hsl]),
            "wpt": np.ascontiguousarray(
                Wp[:, hsl].T.reshape(2, P, E).transpose(1, 0, 2)
            ).astype(ml_dtypes.bfloat16),
            "bqv": np.ascontiguousarray(bq[hsl]),
            "bkv": np.ascontiguousarray(bk[hsl]),
            "mask": mask,
        })

    nc = _get_nc()
    try:
        res = run_bass_kernel_spmd(nc, in_maps, core_ids=list(range(NCORES)),
                                   **_run_kwargs)
    except Exception:
        # transient device hiccups (e.g. NRT_EXEC_UNIT_UNRECOVERABLE) have
        # been observed to clear on retry
        import time
        time.sleep(2.0)
        res = run_bass_kernel_spmd(nc, in_maps, core_ids=list(range(NCORES)),
                                   **_run_kwargs)
    outs = [r["out"].astype(np.float32) for r in res.results]
    y = np.stack([
        outs[0] + outs[1] + outs[2] + outs[3] + bias_eff,
        outs[4] + outs[5] + outs[6] + outs[7] + bias_eff,
    ]).astype(np.float32)
    if _run_kwargs:
        return y, res
    return y


# revision 16
# speedup vs baseline: 1.1173x; 1.0099x over previous
"""Causal attention (B=2, T=2048, E=1024, H=16, D=64) on 8 TRN2 NeuronCores.

Sharding: core c handles batch b = c//4 and local head group hg = c%4
(4 heads, 256 head-dims).  Data parallel over batch, tensor parallel over
heads; the output projection is row-parallel, so each core returns a
partial [T, E] output and the host sums the 4 partials per batch.  The
projection bias AND the v-bias contribution (Wp @ bv) are added on the
host: at = PV(v_raw)/denom + bv exactly, so out = out_dev + (Wp@bv + bp).

Device plan (per core, all-bf16 matmuls with fp32 PSUM accumulation):
  phase A (DMA chase): q_t/k_t [hd, t] for head-pair 0 computed directly
    transposed (lhsT=W.T, rhs=xt, contraction over e), ec-outer with 8
    open PSUM groups so matmuls chase the xt DMA arrivals.  DMA order is
    arrival-ordered: scalar ring [wq0, wk0, wq1, wk1, wv], sync ring
    [xt halves, mask, wp], gpsimd ring [bq, bk] + v-ones memsets.
  phase B: v tiles t0..3 (enough for attention ib0), then block-causal
    attention with v4..15 / head-pair-1 q/k / projection interleaved as
    PE fill between attention tiles.
  attention: per-head score tiles st[j, i] (1 PSUM bank each, 3-deep),
    exp on ScalarE (scale=1/8, no max subtraction), causal mask multiply
    on the block diagonal only, PV accumulation over j with 64
    ones-rows giving the softmax denominator replicated on partitions
    0:63; acc pool is 2-deep per head so the next block's PV never
    waits on the reciprocal+normalize drain.
  tail: the last i-block of head-pair 1 is processed in 384+128 column
    chunks; final projections interleave with the 128-chunk and the
    last copies run on ScalarE and DVE concurrently.
"""

import ml_dtypes
import numpy as np

import concourse.bass as bass
import concourse.tile as tile
from concourse import bacc, mybir
from concourse.bass_utils import run_bass_kernel_spmd

B, T, E = 2, 2048, 1024
H, D = 16, 64
NCORES = 8
GROUPS = 4              # cores per batch (tensor parallel over heads)
HL = H // GROUPS        # 4 local heads per core
HDL = HL * D            # 256 local head dims
P = 128
TQ = 512                # i-block (free dim of score tiles)
JB = 128                # j-block (partition dim of score tiles)
N_TB = T // TQ          # 4
N_EC = E // P           # 8
N_TC = T // P           # 16
XH = 1024               # xt SBUF half-tile columns

F32 = mybir.dt.float32
BF16 = mybir.dt.bfloat16
AF = mybir.ActivationFunctionType


def _build_nc():
    nc = bacc.Bacc("TRN2", target_bir_lowering=False, debug=False)
    xt = nc.dram_tensor("xt", [E, T], BF16, kind="ExternalInput").ap()
    wq0 = nc.dram_tensor("wq0", [P, N_EC, P], BF16, kind="ExternalInput").ap()
    wq1 = nc.dram_tensor("wq1", [P, N_EC, P], BF16, kind="ExternalInput").ap()
    wk0 = nc.dram_tensor("wk0", [P, N_EC, P], BF16, kind="ExternalInput").ap()
    wk1 = nc.dram_tensor("wk1", [P, N_EC, P], BF16, kind="ExternalInput").ap()
    wvt = nc.dram_tensor("wvt", [P, N_EC, HDL], BF16, kind="ExternalInput").ap()
    wpt = nc.dram_tensor("wpt", [P, 2, E], BF16, kind="ExternalInput").ap()
    bqv = nc.dram_tensor("bqv", [HDL], F32, kind="ExternalInput").ap()
    bkv = nc.dram_tensor("bkv", [HDL], F32, kind="ExternalInput").ap()
    maskd = nc.dram_tensor("mask", [GROUPS, JB, TQ], BF16,
                           kind="ExternalInput").ap()
    out = nc.dram_tensor("out", [T, E], BF16, kind="ExternalOutput").ap()

    with tile.TileContext(nc) as tc:
        with (
            tc.tile_pool(name="big", bufs=1) as big,
            tc.tile_pool(name="work", bufs=6) as work,
            tc.tile_pool(name="outp", bufs=3) as outp,
        ):
            # ------- input loads, arrival-ordered across 3 rings ----------
            # scalar ring: first-needed weights in consumption order
            wq_h = []
            wk_h = []
            for hc, (wqd, wkd) in enumerate(((wq0, wk0), (wq1, wk1))):
                wq_t = big.tile([P, N_EC, P], BF16, tag=f"wq{hc}",
                                name=f"wq{hc}")
                wk_t = big.tile([P, N_EC, P], BF16, tag=f"wk{hc}",
                                name=f"wk{hc}")
                wq_h.append(wq_t)
                wk_h.append(wk_t)
            # wq0 leads the sync ring and wk0 the scalar ring so the first
            # q and k matmuls can both start as soon as xt(0,0) lands.
            nc.sync.dma_start(wq_h[0], wq0)
            nc.scalar.dma_start(wk_h[0], wk0)
            nc.scalar.dma_start(wq_h[1], wq1)
            nc.scalar.dma_start(wk_h[1], wk1)
            wv_all = big.tile([P, N_EC, HDL], BF16, tag="wv", name="wv")
            nc.scalar.dma_start(wv_all, wvt)
            # sync ring: xt halves (16 x 256KB), then mask + wp (needed late)
            xt_sb = [big.tile([P, XH], BF16, tag=f"xt{i}", name=f"xt{i}")
                     for i in range(2 * N_EC)]
            for ec in range(N_EC):
                for hf in range(2):
                    nc.sync.dma_start(
                        xt_sb[2 * ec + hf],
                        xt[ec * P:(ec + 1) * P, hf * XH:(hf + 1) * XH])
            mask_sb = big.tile([P, GROUPS, TQ], BF16, tag="mask", name="mask")
            nc.sync.dma_start(mask_sb, maskd.rearrange("d p f -> p d f"))
            wp_all = big.tile([P, 2, E], BF16, tag="wp", name="wp")
            nc.sync.dma_start(wp_all, wpt)
            # gpsimd ring: tiny biases, then v ones-region memsets
            bq_sb = big.tile([P, 2], F32, tag="bq", name="bq")
            nc.gpsimd.dma_start(bq_sb, bqv.rearrange("(c p) -> p c", p=P))
            bk_sb = big.tile([P, 2], F32, tag="bk", name="bk")
            nc.gpsimd.dma_start(bk_sb, bkv.rearrange("(c p) -> p c", p=P))

            q_sb = [big.tile([P, T], BF16, tag=f"q{hc}", name=f"q{hc}")
                    for hc in range(2)]
            k_sb = [big.tile([P, T], BF16, tag=f"k{hc}", name=f"k{hc}")
                    for hc in range(2)]
            at_sb = [big.tile([P, T], BF16, tag=f"at{hc}", name=f"at{hc}")
                     for hc in range(2)]
            v_sb = [big.tile([P, HL, 2 * D], BF16, tag=f"v{t}", name=f"v{t}")
                    for t in range(N_TC)]
            for t_ in range(N_TC):
                nc.gpsimd.memset(v_sb[t_][:, :, 0:D], 1.0)

            def xt_cols(ec, c0, c1):
                hf, off = divmod(c0, XH)
                assert c1 - c0 <= XH - off
                return xt_sb[2 * ec + hf][:, off:off + (c1 - c0)]

            # ------- phase A (bf16): q/k for head-pair 0; ec-outer ---------
            def qk_phase(ph2ps, hc):
                # q matmuls before k per ec (wq0 lands first); k bias-adds
                # on GpSimd so the post-phase DVE drain backlog halves.
                pss = [ph2ps.tile([P, TQ], F32, tag="mm", name="mm")
                       for _ in range(8)]
                for ec in range(N_EC):
                    for wi, w_all in enumerate((wq_h, wk_h)):
                        for tb in range(N_TB):
                            nc.tensor.matmul(
                                pss[tb * 2 + wi],
                                lhsT=w_all[hc][:, ec, :],
                                rhs=xt_cols(ec, tb * TQ, (tb + 1) * TQ),
                                start=(ec == 0), stop=(ec == N_EC - 1))
                # GpSimd cannot read PSUM; ScalarE can and is idle here
                # (no exps yet), so it drains the k halves in parallel.
                for tb in range(N_TB):
                    nc.vector.tensor_scalar_add(
                        q_sb[hc][:, tb * TQ:(tb + 1) * TQ],
                        pss[tb * 2], bq_sb[:, hc:hc + 1])
                    nc.scalar.add(
                        k_sb[hc][:, tb * TQ:(tb + 1) * TQ],
                        pss[tb * 2 + 1], bk_sb[:, hc:hc + 1])

            def v_wave(ph2ps, ts):
                pss = [ph2ps.tile([P, HDL], F32, tag="mm", name="mm")
                       for _ in ts]
                for ec in range(N_EC):
                    for i, t_ in enumerate(ts):
                        nc.tensor.matmul(
                            pss[i],
                            lhsT=xt_cols(ec, t_ * P, (t_ + 1) * P),
                            rhs=wv_all[:, ec, :],
                            start=(ec == 0), stop=(ec == N_EC - 1))
                for i, t_ in enumerate(ts):
                    src = pss[i].rearrange("p (h d) -> p h d", h=HL)
                    if t_ % 2 == 0:
                        nc.vector.tensor_copy(v_sb[t_][:, :, D:2 * D], src)
                    else:
                        nc.scalar.copy(v_sb[t_][:, :, D:2 * D], src)

            # ------- phase 3 (bf16): block-causal attention ----------------
            # One [P, 2, TQ] score tile per j-tile covers both heads: the
            # second K=64 score matmul then carries no PSUM-bank wait, so it
            # issues back-to-back with the first and the PE runs the two
            # 64-row quadrants concurrently.
            def norm_chunk(accs, hp, ib, c0, c1):
                base = ib * TQ
                for h in range(2):
                    rec = work.tile([D, 256], F32, tag="rec", name="rec")
                    nc.vector.reciprocal_approx_fast(
                        rec[:, :c1 - c0], accs[h][0:D, c0:c1])
                    nc.vector.tensor_mul(
                        at_sb[hp][h * D:(h + 1) * D, base + c0:base + c1],
                        accs[h][D:2 * D, c0:c1], rec[:, :c1 - c0])

            def attention(stps, accps, hp, ib, filler=None, norm=True):
                base = ib * TQ
                njb = 4 * ib + 4
                accs = [accps.tile([P, TQ], F32, tag=f"acc{h}",
                                   name=f"acc{h}") for h in range(2)]
                for jb in range(njb):
                    d = jb - 4 * ib                 # >= 0 on block diagonal
                    dd = d * JB if d >= 0 else 0    # local masked-col trim
                    st = stps.tile([P, 2, TQ], F32, tag="st", name="st")
                    pt = work.tile([P, 2, TQ], BF16, tag="pt", name="pt")
                    for h in range(2):
                        pr = slice(h * D, (h + 1) * D)
                        nc.tensor.matmul(
                            st[:, h, dd:],
                            lhsT=q_sb[hp][pr, jb * JB:(jb + 1) * JB],
                            rhs=k_sb[hp][pr, base + dd:base + TQ],
                            start=True, stop=True)
                    nc.scalar.activation(pt[:, :, dd:], st[:, :, dd:],
                                         AF.Exp, scale=0.125)
                    if d >= 0:
                        for h in range(2):
                            nc.vector.tensor_mul(
                                pt[:, h, dd:], pt[:, h, dd:],
                                mask_sb[:, d, dd:])
                    for h in range(2):
                        nc.tensor.matmul(
                            accs[h][:, dd:],
                            lhsT=v_sb[jb][:, 2 * hp + h, :],
                            rhs=pt[:, h, dd:],
                            start=(jb == 0), stop=(jb == njb - 1))
                    if filler and jb % 2 == 1:
                        filler.pop(0)()
                if norm:
                    for c0 in range(0, TQ, 256):
                        norm_chunk(accs, hp, ib, c0, c0 + 256)
                return accs

            # ------- fillers: hc1 q/k, v waves, projection -----------------
            def qk2_group(mmps, tb, wi):
                w_all = (wq_h, wk_h)[wi]
                bias_t = (bq_sb, bk_sb)[wi]
                dst = (q_sb, k_sb)[wi]

                def go():
                    ps = mmps.tile([P, TQ], F32, tag="mm", name="mm")
                    for ec in range(N_EC):
                        nc.tensor.matmul(
                            ps,
                            lhsT=w_all[1][:, ec, :],
                            rhs=xt_cols(ec, tb * TQ, (tb + 1) * TQ),
                            start=(ec == 0), stop=(ec == N_EC - 1))
                    nc.vector.tensor_scalar_add(
                        dst[1][:, tb * TQ:(tb + 1) * TQ], ps, bias_t[:, 1:2])
                return go

            def v_group(mmps, t_):
                def go():
                    ps = mmps.tile([P, HDL], F32, tag="mm", name="mm")
                    for ec in range(N_EC):
                        nc.tensor.matmul(
                            ps,
                            lhsT=xt_cols(ec, t_ * P, (t_ + 1) * P),
                            rhs=wv_all[:, ec, :],
                            start=(ec == 0), stop=(ec == N_EC - 1))
                    nc.vector.tensor_copy(
                        v_sb[t_][:, :, D:2 * D],
                        ps.rearrange("p (h d) -> p h d", h=HL))
                return go

            def proj_t(mmps, t_, copy_eng=None):
                # copy_eng: None -> both PSUM drains on DVE; "scalar" -> both
                # on ScalarE; "mix" -> eb0 on ScalarE, eb1 on DVE (parallel).
                def go():
                    ot = outp.tile([P, E], BF16, tag="ot", name="ot")
                    for eb in range(2):
                        ps = mmps.tile([P, TQ], F32, tag="mm", name="mm")
                        for hc in range(2):
                            nc.tensor.matmul(
                                ps,
                                lhsT=at_sb[hc][:, t_ * P:(t_ + 1) * P],
                                rhs=wp_all[:, hc, eb * TQ:(eb + 1) * TQ],
                                start=(hc == 0), stop=(hc == 1))
                        on_scalar = (copy_eng == "scalar"
                                     or (copy_eng == "mix" and eb == 0))
                        if on_scalar:
                            nc.scalar.copy(ot[:, eb * TQ:(eb + 1) * TQ], ps)
                        else:
                            nc.vector.tensor_copy(
                                ot[:, eb * TQ:(eb + 1) * TQ], ps)
                        nc.sync.dma_start(
                            out[t_ * P:(t_ + 1) * P, eb * TQ:(eb + 1) * TQ],
                            ot[:, eb * TQ:(eb + 1) * TQ])
                return go

            # ------- orchestration -----------------------------------------
            import contextlib
            with tc.tile_pool(name="ph2ps", bufs=8, space="PSUM") as ph2ps:
                qk_phase(ph2ps, 0)
                v_wave(ph2ps, ts=(0, 1, 2, 3))
            _ph34 = contextlib.ExitStack()
            stps = _ph34.enter_context(
                tc.tile_pool(name="stps", bufs=2, space="PSUM"))
            accps = _ph34.enter_context(
                tc.tile_pool(name="accps", bufs=1, space="PSUM"))
            mmps = _ph34.enter_context(
                tc.tile_pool(name="mmps", bufs=2, space="PSUM"))

            def F(*gs):
                return list(gs)

            # interleave the two head pairs at i-block granularity; fillers
            # are emitted between attention j-tiles (1 per 2 tiles) AND at
            # every block boundary, where they hide the acc-drain (norm)
            # latency of the previous block.
            # Emission order is program order on each in-order engine queue,
            # so every filler must be emitted after its producers.
            attention(stps, accps, 0, 0,
                      filler=F(v_group(mmps, 4), v_group(mmps, 5)))
            qk2_group(mmps, 0, 0)()
            qk2_group(mmps, 0, 1)()
            attention(stps, accps, 1, 0,
                      filler=F(v_group(mmps, 6), v_group(mmps, 7)))
            qk2_group(mmps, 1, 0)()
            attention(stps, accps, 0, 1,
                      filler=F(qk2_group(mmps, 1, 1),
                               v_group(mmps, 8), v_group(mmps, 9)))
            v_group(mmps, 10)()
            attention(stps, accps, 1, 1,
                      filler=F(proj_t(mmps, 0), proj_t(mmps, 1),
                               proj_t(mmps, 2), proj_t(mmps, 3)))
            qk2_group(mmps, 2, 0)()
            attention(stps, accps, 0, 2,
                      filler=F(qk2_group(mmps, 2, 1),
                               v_group(mmps, 11), v_group(mmps, 12),
                               v_group(mmps, 13), v_group(mmps, 14)))
            v_group(mmps, 15)()
            attention(stps, accps, 1, 2,
                      filler=F(proj_t(mmps, 4), proj_t(mmps, 5),
                               proj_t(mmps, 6), proj_t(mmps, 7),
                               qk2_group(mmps, 3, 0)))
            proj_t(mmps, 8)()
            attention(stps, accps, 0, 3,
                      filler=F(qk2_group(mmps, 3, 1), proj_t(mmps, 9)))
            proj_t(mmps, 10)()
            accs_last = attention(stps, accps, 1, 3, norm=False,
                                  filler=F(proj_t(mmps, 11)))
            # tail: interleave the last block's norm chunks with the final
            # projections so the serial chain only covers 256 columns.
            norm_chunk(accs_last, 1, 3, 0, 256)
            proj_t(mmps, 12, copy_eng="scalar")()
            norm_chunk(accs_last, 1, 3, 256, TQ)
            proj_t(mmps, 13, copy_eng="scalar")()
            proj_t(mmps, 14, copy_eng="mix")()
            proj_t(mmps, 15, copy_eng="mix")()
            _ph34.close()

    nc.compile()
    return nc


def _make_mask():
    jj = np.arange(JB)[:, None]
    ii = np.arange(TQ)[None, :]
    m = np.zeros((GROUPS, JB, TQ), dtype=np.float32)
    for d in range(GROUPS):
        m[d] = (jj + d * JB <= ii).astype(np.float32)
    return m.astype(ml_dtypes.bfloat16)


_NC = None


def _get_nc():
    global _NC
    if _NC is None:
        _NC = _build_nc()
    return _NC


def _warr(w):
    """W slice [HDL, E] -> SBUF layout [P, N_EC, HDL]: element (p, c, f) =
    W.T[c*P + p, f]."""
    return np.ascontiguousarray(
        w.T.reshape(N_EC, P, HDL).transpose(1, 0, 2)).astype(ml_dtypes.bfloat16)


def kernel(x, Wq, bq, Wk, bk, Wv, bv, Wp, bp, **_run_kwargs):
    x = np.asarray(x, dtype=np.float32)
    Wq = np.asarray(Wq, dtype=np.float32)
    Wk = np.asarray(Wk, dtype=np.float32)
    Wv = np.asarray(Wv, dtype=np.float32)
    Wp = np.asarray(Wp, dtype=np.float32)
    bq = np.asarray(bq, dtype=np.float32)
    bk = np.asarray(bk, dtype=np.float32)
    bv = np.asarray(bv, dtype=np.float32)
    bp = np.asarray(bp, dtype=np.float32)

    mask = _make_mask()
    # at = PV(v_raw)/denom + bv exactly, so the v-bias and projection bias
    # fold into one host-side vector: out += Wp @ bv + bp.
    bias_eff = (bp + Wp @ bv).astype(np.float32)

    in_maps = []
    for c in range(NCORES):
        b, hg = divmod(c, GROUPS)
        hsl = slice(HDL * hg, HDL * (hg + 1))
        wq_a = _warr(Wq[hsl])
        wk_a = _warr(Wk[hsl])
        in_maps.append({
            "xt": np.ascontiguousarray(x[b].T).astype(ml_dtypes.bfloat16),
            "wq0": np.ascontiguousarray(wq_a[:, :, 0:P]),
            "wq1": np.ascontiguousarray(wq_a[:, :, P:2 * P]),
            "wk0": np.ascontiguousarray(wk_a[:, :, 0:P]),
            "wk1": np.ascontiguousarray(wk_a[:, :, P:2 * P]),
            "wvt": _warr(Wv[hsl]),
            "wpt": np.ascontiguousarray(
                Wp[:, hsl].T.reshape(2, P, E).transpose(1, 0, 2)
            ).astype(ml_dtypes.bfloat16),
            "bqv": np.ascontiguousarray(bq[hsl]),
            "bkv": np.ascontiguousarray(bk[hsl]),
            "mask": mask,
        })

    nc = _get_nc()
    try:
        res = run_bass_kernel_spmd(nc, in_maps, core_ids=list(range(NCORES)),
                                   **_run_kwargs)
    except Exception:
        # transient device hiccups (e.g. NRT_EXEC_UNIT_UNRECOVERABLE) have
        # been observed to clear on retry
        import time
        time.sleep(2.0)
        res = run_bass_kernel_spmd(nc, in_maps, core_ids=list(range(NCORES)),
                                   **_run_kwargs)
    outs = [r["out"].astype(np.float32) for r in res.results]
    y = np.stack([
        outs[0] + outs[1] + outs[2] + outs[3] + bias_eff,
        outs[4] + outs[5] + outs[6] + outs[7] + bias_eff,
    ]).astype(np.float32)
    if _run_kwargs:
        return y, res
    return y
